# revision 40
# baseline (speedup 1.0000x reference)
"""Trainium2 Bass kernel for multi-head causal attention with RoPE.

Problem: x[4,2048,1024] -> MHA(16 heads, head_dim 64, RoPE, causal) -> [4,2048,1024]

Sharding: 8 cores = 4 batches x 2 head-groups (8 heads each, Megatron-style).
Each core computes a partial [T, C] projection output for its batch; the host
sums the two head-group partials per batch and adds b_proj.

Per-core dataflow (all on-device), v3:
  - x arrives in DRAM as bf16; x^T materialized by XBAR DMA-transpose straight
    into SBUF (no PE transposes, no PSUM->SBUF copies)
  - Q^T/K^T projected in bf16 ([2 heads x 64d, T] per slice), RoPE via a
    bf16 permutation matmul + cos/sin multiplies (bf16 keeps score noise at
    ~0.5%; fp8 Q/K was measured at 3.1e-2 rel err, over the 2e-2 gate)
  - scores S^T = K Q^T in bf16 (1 cyc/row), [128k, 2x512q] PSUM tiles
  - softmax without max-subtraction: exp(S/8 - 2) fused into one ACT
    activation per tile, fp8e4m3 output; diagonal staircase masking via
    GpSimd memsets + one [128,128] triangular fp8 multiply on DVE
  - P@V and denominator ones-matmuls as fp8 DoubleRow matmuls over 256-key
    superblocks; each head's V stationary is zero-padded to [128, 2, 128]
    (head h in columns 64*(h%2)..) so full-width DR outputs from both heads
    of a pair accumulate into one PSUM bank
  - normalize = one reciprocal + one multiply -> y^T fp32r
  - output projection y^T @ W_proj in fp32r, DVE copy, DMA out
  - schedule: phase B for a (pair, qc) emits ALL score+exp tiles first, then
    all PV/den matmuls (exp latency hidden); A-projection and C-output chunks
    are interleaved between attention batches so the PE stays fed while the
    ACT engine grinds through exps; weight DMAs are split per 128-row slice
    so the first projection starts ~2us after launch
"""

import math
import sys

import numpy as np
import ml_dtypes

if "/opt/trn_rl_repo" not in sys.path:
    sys.path.insert(0, "/opt/trn_rl_repo")

import concourse.bass as bass
import concourse.tile as tile
from concourse import bacc
from concourse import mybir
from concourse.bass_utils import run_bass_kernel_spmd

B, T, C = 4, 2048, 1024
NH, D = 16, 64
HL = 8              # local heads per core
DL = HL * D         # 512
NCORES = 8
P = 128
TCH = 512           # t-chunk width
NTC = T // TCH      # 4
ROPE_BASE = 10000.0

F32 = mybir.dt.float32
F32R = mybir.dt.float32r
BF16 = mybir.dt.bfloat16
F8 = mybir.dt.float8e4
Exp = mybir.ActivationFunctionType.Exp
DR = mybir.MatmulPerfMode.DoubleRow
ADD = mybir.AluOpType.add
SUB = mybir.AluOpType.subtract
MULT = mybir.AluOpType.mult


class _Ctx:
    pass


def _emit(tc, xb16, wqk, wv, wp, cos16, sin16, bias, mask8, perm16,
          out):
    nc = tc.nc
    with tc.tile_pool(name="pers", bufs=1) as pers:
        # bf16 Q^T/K^T: [2 heads x 64d, (Qp0..Qp3,Kp0..Kp3), T]
        qk16 = pers.tile([P, 8, T], BF16)
        # fp8 V, two-term (hi + lo residual, ~0.2% effective precision):
        # [t mod 128, t tile, head slot, 128]; head h=2*pair+hh lives in
        # columns [64*hh, 64*hh+64) of slot h, other half stays zero
        vsb = pers.tile([P, T // P, HL, P], F8)
        vlo = pers.tile([P, 2, HL, P], F8)
        yT = pers.tile([P, 4, T], F32R)
        wqk_sb = pers.tile([P, 8, 2 * DL], BF16)
        wv_sb = pers.tile([P, 8, DL], BF16)
        wp_sb = pers.tile([P, 4, C], F32R)
        cos_sb = pers.tile([P, T], BF16)
        sin_sb = pers.tile([P, T], BF16)
        bias_sb = pers.tile([P, 8 + DL], F32)
        mask_sb = pers.tile([P, P], F8)
        perm_sb = pers.tile([P, P], BF16)
        ones8 = pers.tile([P, 2, 2 * P], F8)
        expb = pers.tile([P, 1], F32)

        with tc.tile_pool(name="sb", bufs=2) as psb, \
             tc.tile_pool(name="ro", bufs=2) as pro, \
             tc.tile_pool(name="pt8", bufs=10) as ppt, \
             tc.tile_pool(name="ost", bufs=3) as post, \
             tc.tile_pool(name="psA", bufs=3, space="PSUM") as psA, \
             tc.tile_pool(name="psS", bufs=2, space="PSUM") as psS, \
             tc.tile_pool(name="psOD", bufs=1, space="PSUM") as psOD:

            ctx = _Ctx()
            ctx.xT16 = {}
            ctx.pending_out = []
            ctx.pts = {}

            def load_consts():
                # ordered for startup latency: first QK chain needs wqk
                # slices + xbar(0) (emitted just before this); RoPE needs
                # cos/sin/bias/perm at ~3us; V needs wv + zeros at ~10us;
                # masks at ~15us; wp only for phase C
                nc.sync.dma_start(
                    wqk_sb[:], wqk.rearrange("(o p) n -> p o n", p=P))
                nc.scalar.dma_start(cos_sb[:], cos16)
                nc.scalar.dma_start(sin_sb[:], sin16)
                nc.scalar.dma_start(bias_sb[:], bias)
                nc.scalar.dma_start(perm_sb[:], perm16)
                nc.scalar.dma_start(mask_sb[:], mask8)
                nc.sync.dma_start(
                    wv_sb[:], wv.rearrange("(o p) n -> p o n", p=P))
                nc.gpsimd.memset(ones8[:], 0.0)
                nc.vector.memset(ones8[:, :, 0:D], 1.0)
                nc.vector.memset(ones8[:, :, 3 * D:4 * D], 1.0)
                nc.vector.memset(expb[:], -2.0)
                nc.gpsimd.dma_start(
                    wp_sb[:], wp.rearrange("(o p) n -> p o n", p=P))

            def a_xbar(tcn, eng=None):
                ts0 = tcn * TCH
                t0 = tcn * (TCH // P)
                nc.gpsimd.memset(vsb[:, t0:t0 + TCH // P], 0.0)
                if tcn == 0:
                    nc.gpsimd.memset(vlo[:], 0.0)
                xT16 = psb.tile([P, 8, TCH], BF16, tag="xT")
                (eng or nc.sync).dma_start_transpose(
                    xT16[:], xb16[ts0:ts0 + TCH, :])
                ctx.xT16[tcn] = xT16

            def a_proj_j(tcn, j):
                ts0 = tcn * TCH
                xT16 = ctx.xT16[tcn]
                psq = psA.tile([P, TCH], F32, tag="a")
                for cc in range(8):
                    nc.tensor.matmul(
                        psq[:],
                        wqk_sb[:, cc, j * P:(j + 1) * P],
                        xT16[:, cc, :],
                        start=(cc == 0), stop=(cc == 7))
                t1 = pro.tile([P, TCH], BF16, tag="t1")
                nc.vector.tensor_scalar_add(t1[:], psq[:], bias_sb[:, j:j + 1])
                psw = psA.tile([P, TCH], F32, tag="a")
                nc.tensor.matmul(psw[:], perm_sb[:], t1[:],
                                 start=True, stop=True)
                dst = qk16[:, j, ts0:ts0 + TCH]
                nc.vector.tensor_tensor(
                    dst, t1[:], cos_sb[:, ts0:ts0 + TCH], MULT)
                swp = pro.tile([P, TCH], BF16, tag="swp")
                nc.vector.tensor_tensor(
                    swp[:], psw[:], sin_sb[:, ts0:ts0 + TCH], MULT)
                nc.gpsimd.tensor_tensor(dst, dst, swp[:], ADD)

            def a_vproj(tcn, i):
                xT16 = ctx.xT16[tcn]
                ti = tcn * (TCH // P) + i
                psv = psA.tile([P, DL], F32, tag="a")
                for cc in range(8):
                    nc.tensor.matmul(
                        psv[:],
                        xT16[:, cc, i * P:(i + 1) * P],
                        wv_sb[:, cc, :],
                        start=(cc == 0), stop=(cc == 7))
                tv = pro.tile([P, DL], F32, tag="tv")
                nc.vector.tensor_tensor(
                    tv[:], psv[:], bias_sb[:, 8:8 + DL], ADD)
                tv3 = tv.rearrange("p (s two e) -> p s two e", two=2, e=D)
                nc.vector.tensor_copy(vsb[:, ti, 0:HL:2, 0:D], tv3[:, :, 0, :])
                nc.vector.tensor_copy(
                    vsb[:, ti, 1:HL:2, D:2 * D], tv3[:, :, 1, :])
                if ti < 2:
                    nc.vector.tensor_tensor(
                        vlo[:, ti, 0:HL:2, 0:D], tv3[:, :, 0, :],
                        vsb[:, ti, 0:HL:2, 0:D], SUB)
                    nc.vector.tensor_tensor(
                        vlo[:, ti, 1:HL:2, D:2 * D], tv3[:, :, 1, :],
                        vsb[:, ti, 1:HL:2, D:2 * D], SUB)

            def b_sc_thunk(qc, pair, k2):
                q0 = qc * 256
                pss = psS.tile([P, 2, 2, 256], F32, tag="pss")
                for hh in range(2):
                    r0 = 64 * hh
                    for b in range(2):
                        kb = 2 * k2 + b
                        # b=0 start zeroes the whole 2KB bank (pending-zero
                        # is bank-granular); b=1 writes into the pre-zeroed
                        # other half as the same group
                        nc.tensor.matmul(
                            pss[:, hh, b, :],
                            qk16[r0:r0 + 64, 4 + pair, kb * P:(kb + 1) * P],
                            qk16[r0:r0 + 64, pair, q0:q0 + 256],
                            start=(b == 0), stop=(b == 1),
                            tile_position=(r0, 0),
                            skip_group_check=True)
                pt = ppt.tile([P, 2, 2, 256], F8, tag="pt")
                nc.scalar.activation(
                    pt[:], pss[:], Exp, bias=expb[:, 0:1], scale=0.125)
                for b in range(2):
                    mm = 2 * k2 + b - 2 * qc
                    if mm < 0:
                        continue
                    if mm > 0:
                        nc.gpsimd.memset(pt[:, 0, b, 0:P], 0.0)
                        nc.gpsimd.memset(pt[:, 1, b, 0:P], 0.0)
                    m0 = P * mm
                    nc.vector.tensor_tensor(
                        pt[:, 0, b, m0:m0 + P], pt[:, 0, b, m0:m0 + P],
                        mask_sb[:], MULT)
                    nc.vector.tensor_tensor(
                        pt[:, 1, b, m0:m0 + P], pt[:, 1, b, m0:m0 + P],
                        mask_sb[:], MULT)
                ctx.pts[(pair, k2)] = pt

            def b_pv_thunk(qc, pair, k2, pod):
                nk2 = qc + 1
                pt2 = ctx.pts.pop((pair, k2))
                for hh in range(2):
                    h = 2 * pair + hh
                    pt = pt2[:, hh]
                    first = k2 == 0 and hh == 0
                    last = k2 == nk2 - 1 and hh == 1
                    use_lo = qc == 0
                    nc.tensor.matmul(
                        pod[:, 0:256],
                        vsb[:, 2 * k2:2 * k2 + 2, h, :],
                        pt[:],
                        start=first, stop=(last and not use_lo),
                        perf_mode=DR, skip_group_check=True)
                    if use_lo:
                        nc.tensor.matmul(
                            pod[:, 0:256],
                            vlo[:, 0:2, h, :],
                            pt[:],
                            start=False, stop=last,
                            perf_mode=DR, skip_group_check=True)
                    # start=False: the first PV-hi's start already marked
                    # this whole bank pending-zero
                    nc.tensor.matmul(
                        pod[:, 256:512],
                        ones8[:, :, P * hh:P * (hh + 1)],
                        pt[:],
                        start=False, stop=last,
                        perf_mode=DR, skip_group_check=True)

            def b_norm_thunk(qc, pair, pod):
                q0 = qc * 256
                rcp = pro.tile([P, 256], F32, tag="rcp")
                nc.vector.reciprocal(rcp[:], pod[:, 256:512])
                nc.vector.tensor_tensor(
                    yT[:, pair, q0:q0 + 256], pod[:, 0:256], rcp[:], MULT)

            def c_out(tcn, i):
                ti = tcn * (TCH // P) + i
                ost = post.tile([P, 1024], F32, tag="ost")
                for n in range(2):
                    psp = psA.tile([P, TCH], F32, tag="a")
                    for g in range(4):
                        nc.tensor.matmul(
                            psp[:],
                            yT[:, g, ti * P:(ti + 1) * P],
                            wp_sb[:, g, n * TCH:(n + 1) * TCH],
                            start=(g == 0), stop=(g == 3))
                    nc.vector.tensor_copy(
                        ost[:, n * TCH:(n + 1) * TCH], psp[:])
                ctx.pending_out.append((ti, ost))

            def c_dma():
                while ctx.pending_out:
                    ti, ost = ctx.pending_out.pop(0)
                    nc.sync.dma_start(out[ti * P:(ti + 1) * P, :], ost[:])

            # ---- schedule ----
            a_xbar(0, eng=nc.scalar)
            load_consts()
            for j in range(8):
                a_proj_j(0, j)
            for i in range(4):
                a_vproj(0, i)
            for seg in range(NTC):
                fills = []
                if seg + 1 < NTC:
                    tn = seg + 1
                    fills.append((lambda tn=tn: a_xbar(tn), 500))
                    for j in range(8):
                        fills.append(
                            (lambda tn=tn, j=j: a_proj_j(tn, j), 1900))
                    for i in range(4):
                        fills.append(
                            (lambda tn=tn, i=i: a_vproj(tn, i), 1800))
                fills.append((c_dma, 100))
                if seg == NTC - 1:
                    for tn in range(NTC - 1):
                        for i in range(4):
                            fills.append(
                                (lambda tn=tn, i=i: c_out(tn, i), 1800))
                        fills.append((c_dma, 100))
                # weave B thunks with fills by ACT-vs-PE deficit
                deficit = 0.0
                fi = 0
                for qh in range(2):
                    qc = 2 * seg + qh
                    nk2 = qc + 1
                    for pair in range(4):
                        pod = psOD.tile([P, 512], F32, tag="od")
                        for k2 in range(nk2):
                            b_sc_thunk(qc, pair, k2)
                            deficit += 611
                            if k2 >= 1:
                                b_pv_thunk(qc, pair, k2 - 1, pod)
                                deficit -= 320
                            while (deficit > 0 and fi < len(fills)):
                                fn, cost = fills[fi]
                                fn()
                                fi += 1
                                deficit -= cost
                        b_pv_thunk(qc, pair, nk2 - 1, pod)
                        b_norm_thunk(qc, pair, pod)
                while fi < len(fills):
                    fills[fi][0]()
                    fi += 1
            for i in range(4):
                c_out(NTC - 1, i)
            c_dma()


def build_nc():
    nc = bacc.Bacc("TRN2", target_bir_lowering=False, debug=False)
    xb16 = nc.dram_tensor("xb16", [T, C], BF16, kind="ExternalInput").ap()
    wqk = nc.dram_tensor("wqk", [C, 2 * DL], BF16, kind="ExternalInput").ap()
    wv = nc.dram_tensor("wv", [C, DL], BF16, kind="ExternalInput").ap()
    wp = nc.dram_tensor("wp", [DL, C], F32, kind="ExternalInput").ap()
    cos16 = nc.dram_tensor("cos16", [P, T], BF16, kind="ExternalInput").ap()
    sin16 = nc.dram_tensor("sin16", [P, T], BF16, kind="ExternalInput").ap()
    bias = nc.dram_tensor("bias", [P, 8 + DL], F32, kind="ExternalInput").ap()
    mask8 = nc.dram_tensor("mask8", [P, P], F8, kind="ExternalInput").ap()
    perm16 = nc.dram_tensor("perm16", [P, P], BF16, kind="ExternalInput").ap()
    out = nc.dram_tensor("out", [T, C], F32, kind="ExternalOutput").ap()
    with tile.TileContext(nc) as tc:
        _emit(tc, xb16, wqk, wv, wp, cos16, sin16, bias, mask8, perm16,
              out)
    nc.compile()
    return nc


def rope_tables():
    inv_freq = 1.0 / (ROPE_BASE ** (np.arange(0, D, 2, dtype=np.float64) / D))
    t = np.arange(T, dtype=np.float64)
    freqs = np.outer(t, inv_freq)                      # [T, 32]
    emb = np.concatenate([freqs, freqs], axis=-1)      # [T, 64]
    cosT = np.cos(emb).T                               # [64, T]
    sinT = np.sin(emb).T
    cos2 = np.tile(cosT, (2, 1)).astype(ml_dtypes.bfloat16)
    sin2 = np.tile(sinT, (2, 1)).astype(ml_dtypes.bfloat16)
    return np.ascontiguousarray(cos2), np.ascontiguousarray(sin2)


def perm_matrix():
    pm = np.zeros((P, P), dtype=np.float32)
    for base in (0, 64):
        for d in range(32):
            pm[base + d + 32, base + d] = -1.0       # rot_half: -x2 into top
            pm[base + d, base + d + 32] = 1.0        # +x1 into bottom
    return np.ascontiguousarray(pm.astype(ml_dtypes.bfloat16))


def tri_mask8():
    k = np.arange(P)[:, None]
    q = np.arange(P)[None, :]
    return np.ascontiguousarray((k <= q).astype(ml_dtypes.float8_e4m3))


def host_inputs(x, W_qkv, b_qkv, W_proj, b_proj):
    x = np.asarray(x, dtype=np.float32)
    W_qkv = np.asarray(W_qkv, dtype=np.float32)
    b_qkv = np.asarray(b_qkv, dtype=np.float32)
    W_proj = np.asarray(W_proj, dtype=np.float32)
    cos2, sin2 = rope_tables()
    mask8 = tri_mask8()
    pm = perm_matrix()
    in_maps = []
    for core in range(NCORES):
        b = core // 2
        hg = core % 2
        s = hg * DL
        wq = W_qkv[:, s:s + DL]
        wk = W_qkv[:, C + s:C + s + DL]
        wqk = np.ascontiguousarray(
            np.concatenate([wq, wk], axis=1).astype(ml_dtypes.bfloat16))
        wv = np.ascontiguousarray(
            W_qkv[:, 2 * C + s:2 * C + s + DL].astype(ml_dtypes.bfloat16))
        wp = np.ascontiguousarray(W_proj[s:s + DL, :])
        bq = b_qkv[s:s + DL]
        bk = b_qkv[C + s:C + s + DL]
        bv = b_qkv[2 * C + s:2 * C + s + DL]
        bqk = np.concatenate([bq, bk]).reshape(8, P).T          # [128, 8]
        bvb = np.tile(bv[None, :], (P, 1))                      # [128, 512]
        bias = np.ascontiguousarray(
            np.concatenate([bqk, bvb], axis=1).astype(np.float32))
        in_maps.append({
            "xb16": np.ascontiguousarray(x[b].astype(ml_dtypes.bfloat16)),
            "wqk": wqk, "wv": wv, "wp": wp,
            "cos16": cos2, "sin16": sin2, "bias": bias, "mask8": mask8,
            "perm16": pm,
        })
    return in_maps


_NC_CACHE = {}


def run(in_maps, **kwargs):
    if "nc" not in _NC_CACHE:
        _NC_CACHE["nc"] = build_nc()
    return run_bass_kernel_spmd(
        _NC_CACHE["nc"], in_maps, core_ids=list(range(NCORES)), **kwargs)


def kernel(x, W_qkv, b_qkv, W_proj, b_proj, **extra):
    in_maps = host_inputs(x, W_qkv, b_qkv, W_proj, b_proj)
    res = run(in_maps)
    b_proj = np.asarray(b_proj, dtype=np.float32)
    out = np.empty((B, T, C), dtype=np.float32)
    for b in range(B):
        out[b] = res.results[2 * b]["out"] + res.results[2 * b + 1]["out"] + b_proj
    return out


# revision 41
# speedup vs baseline: 1.0138x; 1.0138x over previous
"""Trainium2 Bass kernel for multi-head causal attention with RoPE.

Problem: x[4,2048,1024] -> MHA(16 heads, head_dim 64, RoPE, causal) -> [4,2048,1024]

Sharding: 8 cores = 4 batches x 2 head-groups (8 heads each, Megatron-style).
Each core computes a partial [T, C] projection output for its batch; the host
sums the two head-group partials per batch and adds b_proj.

Per-core dataflow (all on-device), v3:
  - x arrives in DRAM as bf16; x^T materialized by XBAR DMA-transpose straight
    into SBUF (no PE transposes, no PSUM->SBUF copies)
  - Q^T/K^T projected in bf16 ([2 heads x 64d, T] per slice), RoPE via a
    bf16 permutation matmul + cos/sin multiplies (bf16 keeps score noise at
    ~0.5%; fp8 Q/K was measured at 3.1e-2 rel err, over the 2e-2 gate)
  - scores S^T = K Q^T in bf16 (1 cyc/row), [128k, 2x512q] PSUM tiles
  - softmax without max-subtraction: exp(S/8 - 2) fused into one ACT
    activation per tile, fp8e4m3 output; diagonal staircase masking via
    GpSimd memsets + one [128,128] triangular fp8 multiply on DVE
  - P@V and denominator ones-matmuls as fp8 DoubleRow matmuls over 256-key
    superblocks; each head's V stationary is zero-padded to [128, 2, 128]
    (head h in columns 64*(h%2)..) so full-width DR outputs from both heads
    of a pair accumulate into one PSUM bank
  - normalize = one reciprocal + one multiply -> y^T fp32r
  - output projection y^T @ W_proj in fp32r, DVE copy, DMA out
  - schedule: phase B for a (pair, qc) emits ALL score+exp tiles first, then
    all PV/den matmuls (exp latency hidden); A-projection and C-output chunks
    are interleaved between attention batches so the PE stays fed while the
    ACT engine grinds through exps; weight DMAs are split per 128-row slice
    so the first projection starts ~2us after launch
"""

import math
import sys

import numpy as np
import ml_dtypes

if "/opt/trn_rl_repo" not in sys.path:
    sys.path.insert(0, "/opt/trn_rl_repo")

import concourse.bass as bass
import concourse.tile as tile
from concourse import bacc
from concourse import mybir
from concourse.bass_utils import run_bass_kernel_spmd

B, T, C = 4, 2048, 1024
NH, D = 16, 64
HL = 8              # local heads per core
DL = HL * D         # 512
NCORES = 8
P = 128
TCH = 512           # t-chunk width
NTC = T // TCH      # 4
ROPE_BASE = 10000.0

F32 = mybir.dt.float32
F32R = mybir.dt.float32r
BF16 = mybir.dt.bfloat16
F8 = mybir.dt.float8e4
Exp = mybir.ActivationFunctionType.Exp
DR = mybir.MatmulPerfMode.DoubleRow
ADD = mybir.AluOpType.add
SUB = mybir.AluOpType.subtract
MULT = mybir.AluOpType.mult


class _Ctx:
    pass


def _emit(tc, xb16, wqk, wv, wp, cos16, sin16, bias, mask8, perm16,
          out):
    nc = tc.nc
    with tc.tile_pool(name="pers", bufs=1) as pers:
        # bf16 Q^T/K^T: [2 heads x 64d, (Qp0..Qp3,Kp0..Kp3), T]
        qk16 = pers.tile([P, 8, T], BF16)
        # fp8 V, two-term (hi + lo residual, ~0.2% effective precision):
        # [t mod 128, t tile, head slot, 128]; head h=2*pair+hh lives in
        # columns [64*hh, 64*hh+64) of slot h, other half stays zero
        vsb = pers.tile([P, T // P, HL, P], F8)
        vlo = pers.tile([P, 2, HL, P], F8)
        yT = pers.tile([P, 4, T], F32R)
        wqk_sb = pers.tile([P, 8, 2 * DL], BF16)
        wv_sb = pers.tile([P, 8, DL], BF16)
        wp_sb = pers.tile([P, 4, C], F32R)
        cos_sb = pers.tile([P, T], BF16)
        sin_sb = pers.tile([P, T], BF16)
        bias_sb = pers.tile([P, 8 + DL], F32)
        mask_sb = pers.tile([P, P], F8)
        perm_sb = pers.tile([P, P], BF16)
        ones8 = pers.tile([P, 2, 2 * P], F8)
        expb = pers.tile([P, 1], F32)

        with tc.tile_pool(name="sb", bufs=2) as psb, \
             tc.tile_pool(name="ro", bufs=2) as pro, \
             tc.tile_pool(name="pt8", bufs=10) as ppt, \
             tc.tile_pool(name="ost", bufs=3) as post, \
             tc.tile_pool(name="psA", bufs=3, space="PSUM") as psA, \
             tc.tile_pool(name="psS", bufs=2, space="PSUM") as psS, \
             tc.tile_pool(name="psOD", bufs=1, space="PSUM") as psOD:

            ctx = _Ctx()
            ctx.xT16 = {}
            ctx.pending_out = []
            ctx.pts = {}

            def load_consts():
                # ordered for startup latency: first QK chain needs wqk
                # slices + xbar(0) (emitted just before this); RoPE needs
                # cos/sin/bias/perm at ~3us; V needs wv + zeros at ~10us;
                # masks at ~15us; wp only for phase C
                nc.sync.dma_start(
                    wqk_sb[:], wqk.rearrange("(o p) n -> p o n", p=P))
                nc.scalar.dma_start(cos_sb[:], cos16)
                nc.scalar.dma_start(sin_sb[:], sin16)
                nc.scalar.dma_start(bias_sb[:], bias)
                nc.scalar.dma_start(perm_sb[:], perm16)
                nc.scalar.dma_start(mask_sb[:], mask8)
                nc.sync.dma_start(
                    wv_sb[:], wv.rearrange("(o p) n -> p o n", p=P))
                nc.gpsimd.memset(ones8[:], 0.0)
                nc.vector.memset(ones8[:, :, 0:D], 1.0)
                nc.vector.memset(ones8[:, :, 3 * D:4 * D], 1.0)
                nc.vector.memset(expb[:], -2.0)
                nc.gpsimd.dma_start(
                    wp_sb[:], wp.rearrange("(o p) n -> p o n", p=P))

            def a_xbar(tcn, eng=None):
                ts0 = tcn * TCH
                t0 = tcn * (TCH // P)
                nc.gpsimd.memset(vsb[:, t0:t0 + TCH // P], 0.0)
                if tcn == 0:
                    nc.gpsimd.memset(vlo[:], 0.0)
                xT16 = psb.tile([P, 8, TCH], BF16, tag="xT")
                (eng or nc.sync).dma_start_transpose(
                    xT16[:], xb16[ts0:ts0 + TCH, :])
                ctx.xT16[tcn] = xT16

            def a_proj_j(tcn, j):
                ts0 = tcn * TCH
                xT16 = ctx.xT16[tcn]
                psq = psA.tile([P, TCH], F32, tag="a")
                for cc in range(8):
                    nc.tensor.matmul(
                        psq[:],
                        wqk_sb[:, cc, j * P:(j + 1) * P],
                        xT16[:, cc, :],
                        start=(cc == 0), stop=(cc == 7))
                t1 = pro.tile([P, TCH], BF16, tag="t1")
                nc.vector.tensor_scalar_add(t1[:], psq[:], bias_sb[:, j:j + 1])
                psw = psA.tile([P, TCH], F32, tag="a")
                nc.tensor.matmul(psw[:], perm_sb[:], t1[:],
                                 start=True, stop=True)
                dst = qk16[:, j, ts0:ts0 + TCH]
                nc.vector.tensor_tensor(
                    dst, t1[:], cos_sb[:, ts0:ts0 + TCH], MULT)
                swp = pro.tile([P, TCH], BF16, tag="swp")
                nc.vector.tensor_tensor(
                    swp[:], psw[:], sin_sb[:, ts0:ts0 + TCH], MULT)
                nc.gpsimd.tensor_tensor(dst, dst, swp[:], ADD)

            def a_vproj(tcn, i):
                xT16 = ctx.xT16[tcn]
                ti = tcn * (TCH // P) + i
                psv = psA.tile([P, DL], F32, tag="a")
                for cc in range(8):
                    nc.tensor.matmul(
                        psv[:],
                        xT16[:, cc, i * P:(i + 1) * P],
                        wv_sb[:, cc, :],
                        start=(cc == 0), stop=(cc == 7))
                tv = pro.tile([P, DL], F32, tag="tv")
                nc.vector.tensor_tensor(
                    tv[:], psv[:], bias_sb[:, 8:8 + DL], ADD)
                tv3 = tv.rearrange("p (s two e) -> p s two e", two=2, e=D)
                nc.vector.tensor_copy(vsb[:, ti, 0:HL:2, 0:D], tv3[:, :, 0, :])
                nc.vector.tensor_copy(
                    vsb[:, ti, 1:HL:2, D:2 * D], tv3[:, :, 1, :])
                if ti < 2:
                    nc.vector.tensor_tensor(
                        vlo[:, ti, 0:HL:2, 0:D], tv3[:, :, 0, :],
                        vsb[:, ti, 0:HL:2, 0:D], SUB)
                    nc.vector.tensor_tensor(
                        vlo[:, ti, 1:HL:2, D:2 * D], tv3[:, :, 1, :],
                        vsb[:, ti, 1:HL:2, D:2 * D], SUB)

            def b_sc_thunk(qc, pair, k2):
                q0 = qc * 256
                pss = psS.tile([P, 2, 2, 256], F32, tag="pss")
                for hh in range(2):
                    r0 = 64 * hh
                    for b in range(2):
                        kb = 2 * k2 + b
                        # b=0 start zeroes the whole 2KB bank (pending-zero
                        # is bank-granular); b=1 writes into the pre-zeroed
                        # other half as the same group
                        nc.tensor.matmul(
                            pss[:, hh, b, :],
                            qk16[r0:r0 + 64, 4 + pair, kb * P:(kb + 1) * P],
                            qk16[r0:r0 + 64, pair, q0:q0 + 256],
                            start=(b == 0), stop=(b == 1),
                            tile_position=(r0, 0),
                            skip_group_check=True)
                pt = ppt.tile([P, 2, 2, 256], F8, tag="pt")
                nc.scalar.activation(
                    pt[:], pss[:], Exp, bias=expb[:, 0:1], scale=0.125)
                for b in range(2):
                    mm = 2 * k2 + b - 2 * qc
                    if mm < 0:
                        continue
                    if mm > 0:
                        nc.gpsimd.memset(pt[:, 0, b, 0:P], 0.0)
                        nc.gpsimd.memset(pt[:, 1, b, 0:P], 0.0)
                    m0 = P * mm
                    nc.vector.tensor_tensor(
                        pt[:, 0, b, m0:m0 + P], pt[:, 0, b, m0:m0 + P],
                        mask_sb[:], MULT)
                    nc.vector.tensor_tensor(
                        pt[:, 1, b, m0:m0 + P], pt[:, 1, b, m0:m0 + P],
                        mask_sb[:], MULT)
                ctx.pts[(pair, k2)] = pt

            def b_pv_thunk(qc, pair, k2, pod):
                nk2 = qc + 1
                pt2 = ctx.pts.pop((pair, k2))
                for hh in range(2):
                    h = 2 * pair + hh
                    pt = pt2[:, hh]
                    first = k2 == 0 and hh == 0
                    last = k2 == nk2 - 1 and hh == 1
                    use_lo = qc == 0
                    nc.tensor.matmul(
                        pod[:, 0:256],
                        vsb[:, 2 * k2:2 * k2 + 2, h, :],
                        pt[:],
                        start=first, stop=(last and not use_lo),
                        perf_mode=DR, skip_group_check=True)
                    if use_lo:
                        nc.tensor.matmul(
                            pod[:, 0:256],
                            vlo[:, 0:2, h, :],
                            pt[:],
                            start=False, stop=last,
                            perf_mode=DR, skip_group_check=True)
                    # start=False: the first PV-hi's start already marked
                    # this whole bank pending-zero
                    nc.tensor.matmul(
                        pod[:, 256:512],
                        ones8[:, :, P * hh:P * (hh + 1)],
                        pt[:],
                        start=False, stop=last,
                        perf_mode=DR, skip_group_check=True)

            def b_norm_thunk(qc, pair, pod):
                q0 = qc * 256
                rcp = pro.tile([P, 256], F32, tag="rcp")
                nc.vector.reciprocal(rcp[:], pod[:, 256:512])
                nc.vector.tensor_tensor(
                    yT[:, pair, q0:q0 + 256], pod[:, 0:256], rcp[:], MULT)

            def c_out(tcn, i):
                ti = tcn * (TCH // P) + i
                ost = post.tile([P, 1024], F32, tag="ost")
                for n in range(2):
                    psp = psA.tile([P, TCH], F32, tag="a")
                    for g in range(4):
                        nc.tensor.matmul(
                            psp[:],
                            yT[:, g, ti * P:(ti + 1) * P],
                            wp_sb[:, g, n * TCH:(n + 1) * TCH],
                            start=(g == 0), stop=(g == 3))
                    nc.vector.tensor_copy(
                        ost[:, n * TCH:(n + 1) * TCH], psp[:])
                ctx.pending_out.append((ti, ost))

            def c_dma():
                while ctx.pending_out:
                    ti, ost = ctx.pending_out.pop(0)
                    nc.sync.dma_start(out[ti * P:(ti + 1) * P, :], ost[:])

            # ---- schedule ----
            a_xbar(0, eng=nc.scalar)
            load_consts()
            for j in range(8):
                a_proj_j(0, j)
            for i in range(4):
                a_vproj(0, i)
            for seg in range(NTC):
                fills = []
                if seg + 1 < NTC:
                    tn = seg + 1
                    fills.append((lambda tn=tn: a_xbar(tn), 500))
                    for j in range(8):
                        fills.append(
                            (lambda tn=tn, j=j: a_proj_j(tn, j), 1900))
                    for i in range(4):
                        fills.append(
                            (lambda tn=tn, i=i: a_vproj(tn, i), 1800))
                fills.append((c_dma, 100))
                if seg == NTC - 1:
                    for tn in range(NTC - 1):
                        for i in range(4):
                            fills.append(
                                (lambda tn=tn, i=i: c_out(tn, i), 1800))
                        fills.append((c_dma, 100))
                # weave B thunks with fills by ACT-vs-PE deficit
                deficit = 0.0
                fi = 0
                for qh in range(2):
                    qc = 2 * seg + qh
                    nk2 = qc + 1
                    for pair in range(4):
                        pod = psOD.tile([P, 512], F32, tag="od")
                        for k2 in range(nk2):
                            b_sc_thunk(qc, pair, k2)
                            deficit += 611
                            while (deficit > 0 and fi < len(fills)):
                                fn, cost = fills[fi]
                                fn()
                                fi += 1
                                deficit -= cost
                        for k2 in range(nk2):
                            b_pv_thunk(qc, pair, k2, pod)
                            deficit -= 320
                        b_norm_thunk(qc, pair, pod)
                while fi < len(fills):
                    fills[fi][0]()
                    fi += 1
            for i in range(4):
                c_out(NTC - 1, i)
            c_dma()


def build_nc():
    nc = bacc.Bacc("TRN2", target_bir_lowering=False, debug=False)
    xb16 = nc.dram_tensor("xb16", [T, C], BF16, kind="ExternalInput").ap()
    wqk = nc.dram_tensor("wqk", [C, 2 * DL], BF16, kind="ExternalInput").ap()
    wv = nc.dram_tensor("wv", [C, DL], BF16, kind="ExternalInput").ap()
    wp = nc.dram_tensor("wp", [DL, C], F32, kind="ExternalInput").ap()
    cos16 = nc.dram_tensor("cos16", [P, T], BF16, kind="ExternalInput").ap()
    sin16 = nc.dram_tensor("sin16", [P, T], BF16, kind="ExternalInput").ap()
    bias = nc.dram_tensor("bias", [P, 8 + DL], F32, kind="ExternalInput").ap()
    mask8 = nc.dram_tensor("mask8", [P, P], F8, kind="ExternalInput").ap()
    perm16 = nc.dram_tensor("perm16", [P, P], BF16, kind="ExternalInput").ap()
    out = nc.dram_tensor("out", [T, C], F32, kind="ExternalOutput").ap()
    with tile.TileContext(nc) as tc:
        _emit(tc, xb16, wqk, wv, wp, cos16, sin16, bias, mask8, perm16,
              out)
    nc.compile()
    return nc


def rope_tables():
    inv_freq = 1.0 / (ROPE_BASE ** (np.arange(0, D, 2, dtype=np.float64) / D))
    t = np.arange(T, dtype=np.float64)
    freqs = np.outer(t, inv_freq)                      # [T, 32]
    emb = np.concatenate([freqs, freqs], axis=-1)      # [T, 64]
    cosT = np.cos(emb).T                               # [64, T]
    sinT = np.sin(emb).T
    cos2 = np.tile(cosT, (2, 1)).astype(ml_dtypes.bfloat16)
    sin2 = np.tile(sinT, (2, 1)).astype(ml_dtypes.bfloat16)
    return np.ascontiguousarray(cos2), np.ascontiguousarray(sin2)


def perm_matrix():
    pm = np.zeros((P, P), dtype=np.float32)
    for base in (0, 64):
        for d in range(32):
            pm[base + d + 32, base + d] = -1.0       # rot_half: -x2 into top
            pm[base + d, base + d + 32] = 1.0        # +x1 into bottom
    return np.ascontiguousarray(pm.astype(ml_dtypes.bfloat16))


def tri_mask8():
    k = np.arange(P)[:, None]
    q = np.arange(P)[None, :]
    return np.ascontiguousarray((k <= q).astype(ml_dtypes.float8_e4m3))


def host_inputs(x, W_qkv, b_qkv, W_proj, b_proj):
    x = np.asarray(x, dtype=np.float32)
    W_qkv = np.asarray(W_qkv, dtype=np.float32)
    b_qkv = np.asarray(b_qkv, dtype=np.float32)
    W_proj = np.asarray(W_proj, dtype=np.float32)
    cos2, sin2 = rope_tables()
    mask8 = tri_mask8()
    pm = perm_matrix()
    in_maps = []
    for core in range(NCORES):
        b = core // 2
        hg = core % 2
        s = hg * DL
        wq = W_qkv[:, s:s + DL]
        wk = W_qkv[:, C + s:C + s + DL]
        wqk = np.ascontiguousarray(
            np.concatenate([wq, wk], axis=1).astype(ml_dtypes.bfloat16))
        wv = np.ascontiguousarray(
            W_qkv[:, 2 * C + s:2 * C + s + DL].astype(ml_dtypes.bfloat16))
        wp = np.ascontiguousarray(W_proj[s:s + DL, :])
        bq = b_qkv[s:s + DL]
        bk = b_qkv[C + s:C + s + DL]
        bv = b_qkv[2 * C + s:2 * C + s + DL]
        bqk = np.concatenate([bq, bk]).reshape(8, P).T          # [128, 8]
        bvb = np.tile(bv[None, :], (P, 1))                      # [128, 512]
        bias = np.ascontiguousarray(
            np.concatenate([bqk, bvb], axis=1).astype(np.float32))
        in_maps.append({
            "xb16": np.ascontiguousarray(x[b].astype(ml_dtypes.bfloat16)),
            "wqk": wqk, "wv": wv, "wp": wp,
            "cos16": cos2, "sin16": sin2, "bias": bias, "mask8": mask8,
            "perm16": pm,
        })
    return in_maps


_NC_CACHE = {}


def run(in_maps, **kwargs):
    if "nc" not in _NC_CACHE:
        _NC_CACHE["nc"] = build_nc()
    return run_bass_kernel_spmd(
        _NC_CACHE["nc"], in_maps, core_ids=list(range(NCORES)), **kwargs)


def kernel(x, W_qkv, b_qkv, W_proj, b_proj, **extra):
    in_maps = host_inputs(x, W_qkv, b_qkv, W_proj, b_proj)
    res = run(in_maps)
    b_proj = np.asarray(b_proj, dtype=np.float32)
    out = np.empty((B, T, C), dtype=np.float32)
    for b in range(B):
        out[b] = res.results[2 * b]["out"] + res.results[2 * b + 1]["out"] + b_proj
    return out


# revision 42
# speedup vs baseline: 1.0221x; 1.0082x over previous
"""Trainium2 Bass kernel for multi-head causal attention with RoPE.

Problem: x[4,2048,1024] -> MHA(16 heads, head_dim 64, RoPE, causal) -> [4,2048,1024]

Sharding: 8 cores = 4 batches x 2 head-groups (8 heads each, Megatron-style).
Each core computes a partial [T, C] projection output for its batch; the host
sums the two head-group partials per batch and adds b_proj.

Per-core dataflow (all on-device), v3:
  - x arrives in DRAM as bf16; x^T materialized by XBAR DMA-transpose straight
    into SBUF (no PE transposes, no PSUM->SBUF copies)
  - Q^T/K^T projected in bf16 ([2 heads x 64d, T] per slice), RoPE via a
    bf16 permutation matmul + cos/sin multiplies (bf16 keeps score noise at
    ~0.5%; fp8 Q/K was measured at 3.1e-2 rel err, over the 2e-2 gate)
  - scores S^T = K Q^T in bf16 (1 cyc/row), [128k, 2x512q] PSUM tiles
  - softmax without max-subtraction: exp(S/8 - 2) fused into one ACT
    activation per tile, fp8e4m3 output; diagonal staircase masking via
    GpSimd memsets + one [128,128] triangular fp8 multiply on DVE
  - P@V and denominator ones-matmuls as fp8 DoubleRow matmuls over 256-key
    superblocks; each head's V stationary is zero-padded to [128, 2, 128]
    (head h in columns 64*(h%2)..) so full-width DR outputs from both heads
    of a pair accumulate into one PSUM bank
  - normalize = one reciprocal + one multiply -> y^T fp32r
  - output projection y^T @ W_proj in fp32r, DVE copy, DMA out
  - schedule: phase B for a (pair, qc) emits ALL score+exp tiles first, then
    all PV/den matmuls (exp latency hidden); A-projection and C-output chunks
    are interleaved between attention batches so the PE stays fed while the
    ACT engine grinds through exps; weight DMAs are split per 128-row slice
    so the first projection starts ~2us after launch
"""

import math
import sys

import numpy as np
import ml_dtypes

if "/opt/trn_rl_repo" not in sys.path:
    sys.path.insert(0, "/opt/trn_rl_repo")

import concourse.bass as bass
import concourse.tile as tile
from concourse import bacc
from concourse import mybir
from concourse.bass_utils import run_bass_kernel_spmd

B, T, C = 4, 2048, 1024
NH, D = 16, 64
HL = 8              # local heads per core
DL = HL * D         # 512
NCORES = 8
P = 128
TCH = 512           # t-chunk width
NTC = T // TCH      # 4
ROPE_BASE = 10000.0

F32 = mybir.dt.float32
F32R = mybir.dt.float32r
BF16 = mybir.dt.bfloat16
F8 = mybir.dt.float8e4
Exp = mybir.ActivationFunctionType.Exp
DR = mybir.MatmulPerfMode.DoubleRow
ADD = mybir.AluOpType.add
SUB = mybir.AluOpType.subtract
MULT = mybir.AluOpType.mult


class _Ctx:
    pass


def _emit(tc, xb16, wqk, wv, wp, cos16, sin16, bias, mask8, perm16,
          out):
    nc = tc.nc
    with tc.tile_pool(name="pers", bufs=1) as pers:
        # bf16 Q^T/K^T: [2 heads x 64d, (Qp0..Qp3,Kp0..Kp3), T]
        qk16 = pers.tile([P, 8, T], BF16)
        # fp8 V, two-term (hi + lo residual, ~0.2% effective precision):
        # [t mod 128, t tile, head slot, 128]; head h=2*pair+hh lives in
        # columns [64*hh, 64*hh+64) of slot h, other half stays zero
        vsb = pers.tile([P, T // P, HL, P], F8)
        vlo = pers.tile([P, 2, HL, P], F8)
        yT = pers.tile([P, 4, T], F32R)
        wqk_sb = pers.tile([P, 8, 2 * DL], BF16)
        wv_sb = pers.tile([P, 8, DL], BF16)
        wp_sb = pers.tile([P, 4, C], F32R)
        cos_sb = pers.tile([P, T], BF16)
        sin_sb = pers.tile([P, T], BF16)
        bias_sb = pers.tile([P, 8 + DL], F32)
        mask_sb = pers.tile([P, P], F8)
        perm_sb = pers.tile([P, P], BF16)
        ones8 = pers.tile([P, 2, 2 * P], F8)
        expb = pers.tile([P, 1], F32)

        with tc.tile_pool(name="sb", bufs=2) as psb, \
             tc.tile_pool(name="ro", bufs=2) as pro, \
             tc.tile_pool(name="pt8", bufs=10) as ppt, \
             tc.tile_pool(name="ost", bufs=3) as post, \
             tc.tile_pool(name="psA", bufs=3, space="PSUM") as psA, \
             tc.tile_pool(name="psS", bufs=2, space="PSUM") as psS, \
             tc.tile_pool(name="psOD", bufs=1, space="PSUM") as psOD:

            ctx = _Ctx()
            ctx.xT16 = {}
            ctx.pending_out = []
            ctx.pts = {}

            def load_consts():
                # ordered for startup latency: first QK chain needs wqk
                # slices + xbar(0) (emitted just before this); RoPE needs
                # cos/sin/bias/perm at ~3us; V needs wv + zeros at ~10us;
                # masks at ~15us; wp only for phase C
                nc.sync.dma_start(
                    wqk_sb[:], wqk.rearrange("(o p) n -> p o n", p=P))
                nc.scalar.dma_start(cos_sb[:], cos16)
                nc.scalar.dma_start(sin_sb[:], sin16)
                nc.scalar.dma_start(bias_sb[:], bias)
                nc.scalar.dma_start(perm_sb[:], perm16)
                nc.scalar.dma_start(mask_sb[:], mask8)
                nc.sync.dma_start(
                    wv_sb[:], wv.rearrange("(o p) n -> p o n", p=P))
                nc.gpsimd.memset(ones8[:], 0.0)
                nc.vector.memset(ones8[:, :, 0:D], 1.0)
                nc.vector.memset(ones8[:, :, 3 * D:4 * D], 1.0)
                nc.vector.memset(expb[:], -2.0)
                nc.gpsimd.dma_start(
                    wp_sb[:], wp.rearrange("(o p) n -> p o n", p=P))

            def a_xbar(tcn, eng=None):
                ts0 = tcn * TCH
                t0 = tcn * (TCH // P)
                nc.gpsimd.memset(vsb[:, t0:t0 + TCH // P], 0.0)
                if tcn == 0:
                    nc.gpsimd.memset(vlo[:], 0.0)
                xT16 = psb.tile([P, 8, TCH], BF16, tag="xT")
                (eng or nc.sync).dma_start_transpose(
                    xT16[:], xb16[ts0:ts0 + TCH, :])
                ctx.xT16[tcn] = xT16

            def a_proj_j(tcn, j):
                ts0 = tcn * TCH
                xT16 = ctx.xT16[tcn]
                psq = psA.tile([P, TCH], F32, tag="a")
                for cc in range(8):
                    nc.tensor.matmul(
                        psq[:],
                        wqk_sb[:, cc, j * P:(j + 1) * P],
                        xT16[:, cc, :],
                        start=(cc == 0), stop=(cc == 7))
                t1 = pro.tile([P, TCH], BF16, tag="t1")
                nc.vector.tensor_scalar_add(t1[:], psq[:], bias_sb[:, j:j + 1])
                psw = psA.tile([P, TCH], F32, tag="a")
                nc.tensor.matmul(psw[:], perm_sb[:], t1[:],
                                 start=True, stop=True)
                dst = qk16[:, j, ts0:ts0 + TCH]
                nc.vector.tensor_tensor(
                    dst, t1[:], cos_sb[:, ts0:ts0 + TCH], MULT)
                swp = pro.tile([P, TCH], BF16, tag="swp")
                nc.vector.tensor_tensor(
                    swp[:], psw[:], sin_sb[:, ts0:ts0 + TCH], MULT)
                nc.gpsimd.tensor_tensor(dst, dst, swp[:], ADD)

            def a_vproj(tcn, i):
                xT16 = ctx.xT16[tcn]
                ti = tcn * (TCH // P) + i
                psv = psA.tile([P, DL], F32, tag="a")
                for cc in range(8):
                    nc.tensor.matmul(
                        psv[:],
                        xT16[:, cc, i * P:(i + 1) * P],
                        wv_sb[:, cc, :],
                        start=(cc == 0), stop=(cc == 7))
                tv = pro.tile([P, DL], F32, tag="tv")
                nc.vector.tensor_tensor(
                    tv[:], psv[:], bias_sb[:, 8:8 + DL], ADD)
                tv3 = tv.rearrange("p (s two e) -> p s two e", two=2, e=D)
                nc.vector.tensor_copy(vsb[:, ti, 0:HL:2, 0:D], tv3[:, :, 0, :])
                nc.vector.tensor_copy(
                    vsb[:, ti, 1:HL:2, D:2 * D], tv3[:, :, 1, :])
                if ti < 2:
                    nc.vector.tensor_tensor(
                        vlo[:, ti, 0:HL:2, 0:D], tv3[:, :, 0, :],
                        vsb[:, ti, 0:HL:2, 0:D], SUB)
                    nc.vector.tensor_tensor(
                        vlo[:, ti, 1:HL:2, D:2 * D], tv3[:, :, 1, :],
                        vsb[:, ti, 1:HL:2, D:2 * D], SUB)

            def b_sc_thunk(qc, pair, k2):
                q0 = qc * 256
                pss = psS.tile([P, 2, 2, 256], F32, tag="pss")
                for hh in range(2):
                    r0 = 64 * hh
                    for b in range(2):
                        kb = 2 * k2 + b
                        # b=0 start zeroes the whole 2KB bank (pending-zero
                        # is bank-granular); b=1 writes into the pre-zeroed
                        # other half as the same group
                        nc.tensor.matmul(
                            pss[:, hh, b, :],
                            qk16[r0:r0 + 64, 4 + pair, kb * P:(kb + 1) * P],
                            qk16[r0:r0 + 64, pair, q0:q0 + 256],
                            start=(b == 0), stop=(b == 1),
                            tile_position=(r0, 0),
                            skip_group_check=True)
                pt = ppt.tile([P, 2, 2, 256], F8, tag="pt")
                nc.scalar.activation(
                    pt[:], pss[:], Exp, bias=expb[:, 0:1], scale=0.125)
                for b in range(2):
                    mm = 2 * k2 + b - 2 * qc
                    if mm < 0:
                        continue
                    if mm > 0:
                        nc.gpsimd.memset(pt[:, 0, b, 0:P], 0.0)
                        nc.gpsimd.memset(pt[:, 1, b, 0:P], 0.0)
                    m0 = P * mm
                    nc.vector.tensor_tensor(
                        pt[:, 0, b, m0:m0 + P], pt[:, 0, b, m0:m0 + P],
                        mask_sb[:], MULT)
                    nc.vector.tensor_tensor(
                        pt[:, 1, b, m0:m0 + P], pt[:, 1, b, m0:m0 + P],
                        mask_sb[:], MULT)
                ctx.pts[(pair, k2)] = pt

            def b_pv_thunk(qc, pair, k2, pod):
                nk2 = qc + 1
                pt2 = ctx.pts.pop((pair, k2))
                for hh in range(2):
                    h = 2 * pair + hh
                    pt = pt2[:, hh]
                    first = k2 == 0 and hh == 0
                    last = k2 == nk2 - 1 and hh == 1
                    use_lo = qc == 0
                    nc.tensor.matmul(
                        pod[:, 0:256],
                        vsb[:, 2 * k2:2 * k2 + 2, h, :],
                        pt[:],
                        start=first, stop=(last and not use_lo),
                        perf_mode=DR, skip_group_check=True)
                    if use_lo:
                        nc.tensor.matmul(
                            pod[:, 0:256],
                            vlo[:, 0:2, h, :],
                            pt[:],
                            start=False, stop=last,
                            perf_mode=DR, skip_group_check=True)
                    # start=False: the first PV-hi's start already marked
                    # this whole bank pending-zero
                    nc.tensor.matmul(
                        pod[:, 256:512],
                        ones8[:, :, P * hh:P * (hh + 1)],
                        pt[:],
                        start=False, stop=last,
                        perf_mode=DR, skip_group_check=True)

            def b_norm_thunk(qc, pair, pod):
                q0 = qc * 256
                rcp = pro.tile([P, 256], F32, tag="rcp")
                nc.vector.reciprocal(rcp[:], pod[:, 256:512])
                nc.vector.tensor_tensor(
                    yT[:, pair, q0:q0 + 256], pod[:, 0:256], rcp[:], MULT)

            def c_out(tcn, i):
                ti = tcn * (TCH // P) + i
                ost = post.tile([P, 1024], F32, tag="ost")
                for n in range(2):
                    psp = psA.tile([P, TCH], F32, tag="a")
                    for g in range(4):
                        nc.tensor.matmul(
                            psp[:],
                            yT[:, g, ti * P:(ti + 1) * P],
                            wp_sb[:, g, n * TCH:(n + 1) * TCH],
                            start=(g == 0), stop=(g == 3))
                    nc.vector.tensor_copy(
                        ost[:, n * TCH:(n + 1) * TCH], psp[:])
                ctx.pending_out.append((ti, ost))

            def c_dma():
                while ctx.pending_out:
                    ti, ost = ctx.pending_out.pop(0)
                    nc.sync.dma_start(out[ti * P:(ti + 1) * P, :], ost[:])

            # ---- schedule ----
            a_xbar(0, eng=nc.scalar)
            load_consts()
            for j in range(8):
                a_proj_j(0, j)
            for i in range(4):
                a_vproj(0, i)
            for seg in range(NTC):
                fills = []
                if seg + 1 < NTC:
                    tn = seg + 1
                    fills.append((lambda tn=tn: a_xbar(tn), 500))
                    for j in range(8):
                        fills.append(
                            (lambda tn=tn, j=j: a_proj_j(tn, j), 1900))
                    for i in range(4):
                        fills.append(
                            (lambda tn=tn, i=i: a_vproj(tn, i), 1800))
                fills.append((c_dma, 100))
                if seg == NTC - 1:
                    for tn in range(NTC - 1):
                        for i in range(4):
                            fills.append(
                                (lambda tn=tn, i=i: c_out(tn, i), 1800))
                        fills.append((c_dma, 100))
                # proportional fill interleave between batches
                fi = 0
                bi = 0
                nb = 16
                for qh in range(2):
                    qc = 2 * seg + qh
                    nk2 = qc + 1
                    for pair in range(4):
                        pod = psOD.tile([P, 512], F32, tag="od")
                        for k2 in range(nk2):
                            b_sc_thunk(qc, pair, k2)
                        bi += 1
                        take = (len(fills) * bi + nb - 1) // nb
                        while fi < take:
                            fills[fi][0]()
                            fi += 1
                        for k2 in range(nk2):
                            b_pv_thunk(qc, pair, k2, pod)
                        b_norm_thunk(qc, pair, pod)
                        bi += 1
                        take = (len(fills) * bi + nb - 1) // nb
                        while fi < take:
                            fills[fi][0]()
                            fi += 1
                while fi < len(fills):
                    fills[fi][0]()
                    fi += 1
            for i in range(4):
                c_out(NTC - 1, i)
            c_dma()


def build_nc():
    nc = bacc.Bacc("TRN2", target_bir_lowering=False, debug=False)
    xb16 = nc.dram_tensor("xb16", [T, C], BF16, kind="ExternalInput").ap()
    wqk = nc.dram_tensor("wqk", [C, 2 * DL], BF16, kind="ExternalInput").ap()
    wv = nc.dram_tensor("wv", [C, DL], BF16, kind="ExternalInput").ap()
    wp = nc.dram_tensor("wp", [DL, C], F32, kind="ExternalInput").ap()
    cos16 = nc.dram_tensor("cos16", [P, T], BF16, kind="ExternalInput").ap()
    sin16 = nc.dram_tensor("sin16", [P, T], BF16, kind="ExternalInput").ap()
    bias = nc.dram_tensor("bias", [P, 8 + DL], F32, kind="ExternalInput").ap()
    mask8 = nc.dram_tensor("mask8", [P, P], F8, kind="ExternalInput").ap()
    perm16 = nc.dram_tensor("perm16", [P, P], BF16, kind="ExternalInput").ap()
    out = nc.dram_tensor("out", [T, C], F32, kind="ExternalOutput").ap()
    with tile.TileContext(nc) as tc:
        _emit(tc, xb16, wqk, wv, wp, cos16, sin16, bias, mask8, perm16,
              out)
    nc.compile()
    return nc


def rope_tables():
    inv_freq = 1.0 / (ROPE_BASE ** (np.arange(0, D, 2, dtype=np.float64) / D))
    t = np.arange(T, dtype=np.float64)
    freqs = np.outer(t, inv_freq)                      # [T, 32]
    emb = np.concatenate([freqs, freqs], axis=-1)      # [T, 64]
    cosT = np.cos(emb).T                               # [64, T]
    sinT = np.sin(emb).T
    cos2 = np.tile(cosT, (2, 1)).astype(ml_dtypes.bfloat16)
    sin2 = np.tile(sinT, (2, 1)).astype(ml_dtypes.bfloat16)
    return np.ascontiguousarray(cos2), np.ascontiguousarray(sin2)


def perm_matrix():
    pm = np.zeros((P, P), dtype=np.float32)
    for base in (0, 64):
        for d in range(32):
            pm[base + d + 32, base + d] = -1.0       # rot_half: -x2 into top
            pm[base + d, base + d + 32] = 1.0        # +x1 into bottom
    return np.ascontiguousarray(pm.astype(ml_dtypes.bfloat16))


def tri_mask8():
    k = np.arange(P)[:, None]
    q = np.arange(P)[None, :]
    return np.ascontiguousarray((k <= q).astype(ml_dtypes.float8_e4m3))


def host_inputs(x, W_qkv, b_qkv, W_proj, b_proj):
    x = np.asarray(x, dtype=np.float32)
    W_qkv = np.asarray(W_qkv, dtype=np.float32)
    b_qkv = np.asarray(b_qkv, dtype=np.float32)
    W_proj = np.asarray(W_proj, dtype=np.float32)
    cos2, sin2 = rope_tables()
    mask8 = tri_mask8()
    pm = perm_matrix()
    in_maps = []
    for core in range(NCORES):
        b = core // 2
        hg = core % 2
        s = hg * DL
        wq = W_qkv[:, s:s + DL]
        wk = W_qkv[:, C + s:C + s + DL]
        wqk = np.ascontiguousarray(
            np.concatenate([wq, wk], axis=1).astype(ml_dtypes.bfloat16))
        wv = np.ascontiguousarray(
            W_qkv[:, 2 * C + s:2 * C + s + DL].astype(ml_dtypes.bfloat16))
        wp = np.ascontiguousarray(W_proj[s:s + DL, :])
        bq = b_qkv[s:s + DL]
        bk = b_qkv[C + s:C + s + DL]
        bv = b_qkv[2 * C + s:2 * C + s + DL]
        bqk = np.concatenate([bq, bk]).reshape(8, P).T          # [128, 8]
        bvb = np.tile(bv[None, :], (P, 1))                      # [128, 512]
        bias = np.ascontiguousarray(
            np.concatenate([bqk, bvb], axis=1).astype(np.float32))
        in_maps.append({
            "xb16": np.ascontiguousarray(x[b].astype(ml_dtypes.bfloat16)),
            "wqk": wqk, "wv": wv, "wp": wp,
            "cos16": cos2, "sin16": sin2, "bias": bias, "mask8": mask8,
            "perm16": pm,
        })
    return in_maps


_NC_CACHE = {}


def run(in_maps, **kwargs):
    if "nc" not in _NC_CACHE:
        _NC_CACHE["nc"] = build_nc()
    return run_bass_kernel_spmd(
        _NC_CACHE["nc"], in_maps, core_ids=list(range(NCORES)), **kwargs)


def kernel(x, W_qkv, b_qkv, W_proj, b_proj, **extra):
    in_maps = host_inputs(x, W_qkv, b_qkv, W_proj, b_proj)
    res = run(in_maps)
    b_proj = np.asarray(b_proj, dtype=np.float32)
    out = np.empty((B, T, C), dtype=np.float32)
    for b in range(B):
        out[b] = res.results[2 * b]["out"] + res.results[2 * b + 1]["out"] + b_proj
    return out
